# revision 5
# baseline (speedup 1.0000x reference)
"""Trainium2 Bass kernel for the hypernet-MoE model (nn_BaseModel_53455162966557).

Math (per sample b):
    h  = relu(relu(x @ W0 + b0) @ W1 + b1)                    [B, D]
    c  = relu(context @ Wh1 + bh1)                            [B, H]
    flat = c @ Wh2 + bh2                                      [B, NPARAMS]
    z  = relu(einsum(h, flat[:, :i0] as [D, M]) + flat[:, i0:i1])
    z2 = einsum(z, flat[:, i1:i2] as [M, D]) + flat[:, i2:]
    out = relu(h + z2)

Key restructuring: flat is never materialized.  For each hypernet unit k,
    A1_k = h @ Wh2[k, :i0].reshape(D, M)        (tensor engine, bf16)
    z    = relu(sum_k c[:, k] * A1_k + h @ Bh2W1 + b1m)
and symmetrically for the second per-sample layer with zT.  Wh2 (135 MB)
streams through SBUF exactly once per core (cast to bf16 in the DMA).

The per-k scale+accumulate is split across three engines so no single one
binds: for each k, one batch-half uses a fused DVE scalar_tensor_tensor
(fp32), the other uses an ACT per-partition-scale copy to a bf16 tmp
followed by a PE identity-matmul accumulate into a PSUM accumulator.
The halves rotate with k parity.

Sharding: pure data parallel, batch 2048 -> 8 cores x 256.
"""

import numpy as np

import concourse.bass as bass
import concourse.tile as tile
from concourse import bacc, mybir
from concourse.masks import make_identity

F32 = mybir.dt.float32
BF16 = mybir.dt.bfloat16
AF = mybir.ActivationFunctionType
ALU = mybir.AluOpType

B, OBS, CTX, D, M, H = 2048, 64, 16, 256, 256, 256
NCORES = 8
BS = B // NCORES  # 256 rows per core
I0 = D * M  # 65536
I1 = I0 + M  # 65792
I2 = I1 + M * D  # 131328
NPARAMS = I2 + D  # 131584
G = 8  # Wh2 rows (k values) per DMA group; each dc-half DMA is G*128 KiB src
NGROUPS = H // G

_CACHED_NC = None


def build_nc():
    nc = bacc.Bacc("TRN2", target_bir_lowering=False, debug=False)

    x = nc.dram_tensor("x", [BS, OBS], F32, kind="ExternalInput")
    ctx_in = nc.dram_tensor("context", [BS, CTX], F32, kind="ExternalInput")
    W0 = nc.dram_tensor("W0", [OBS, D], F32, kind="ExternalInput")
    b0 = nc.dram_tensor("b0", [D], F32, kind="ExternalInput")
    W1 = nc.dram_tensor("W1", [D, D], F32, kind="ExternalInput")
    b1 = nc.dram_tensor("b1", [D], F32, kind="ExternalInput")
    Wh1 = nc.dram_tensor("Wh1", [CTX, H], F32, kind="ExternalInput")
    bh1 = nc.dram_tensor("bh1", [H], F32, kind="ExternalInput")
    Wh2 = nc.dram_tensor("Wh2", [H, NPARAMS], F32, kind="ExternalInput")
    bh2 = nc.dram_tensor("bh2", [NPARAMS], F32, kind="ExternalInput")
    out = nc.dram_tensor("out", [BS, D], F32, kind="ExternalOutput")

    with tile.TileContext(nc) as tc:
        with (
            tc.tile_pool(name="consts", bufs=1) as consts,
            tc.tile_pool(name="wts", bufs=1) as wts,
            tc.tile_pool(name="acts", bufs=1) as acts,
            tc.tile_pool(name="accs", bufs=1) as accs,
            tc.tile_pool(name="tmps", bufs=4) as tmps,
            tc.tile_pool(name="wh2s", bufs=4) as wh2s,
            tc.tile_pool(name="pk", bufs=5, space="PSUM") as pkp,
            tc.tile_pool(name="pacc", bufs=1, space="PSUM") as paccp,
            tc.tile_pool(name="pmisc", bufs=2, space="PSUM") as pmisc,
        ):
            # ---- input activations first: they head the critical path ----
            xsb = acts.tile([128, 2, OBS], F32)
            nc.sync.dma_start(out=xsb[:], in_=x[:, :].rearrange("(hb p) o -> p hb o", p=128))
            ctxsb = acts.tile([128, 2, CTX], F32)
            nc.sync.dma_start(out=ctxsb[:], in_=ctx_in[:, :].rearrange("(hb p) o -> p hb o", p=128))
            W0sb = wts.tile([OBS, D], F32)
            nc.sync.dma_start(out=W0sb[:], in_=W0[:])
            W1sb = wts.tile([128, 2, D], F32)
            nc.sync.dma_start(out=W1sb[:], in_=W1[:, :].rearrange("(cc p) j -> p cc j", p=128))
            Wh1sb = wts.tile([CTX, H], F32)
            nc.sync.dma_start(out=Wh1sb[:], in_=Wh1[:])
            b0sb = wts.tile([128, 2], F32)
            nc.sync.dma_start(out=b0sb[:], in_=b0[:].rearrange("(cc p) -> p cc", p=128))
            b1sb = wts.tile([128, 2], F32)
            nc.sync.dma_start(out=b1sb[:], in_=b1[:].rearrange("(cc p) -> p cc", p=128))
            bh1sb = wts.tile([128, 2], F32)
            nc.sync.dma_start(out=bh1sb[:], in_=bh1[:].rearrange("(cc p) -> p cc", p=128))

            ident = consts.tile([128, 128], F32)
            make_identity(nc, ident[:])
            identb = consts.tile([128, 128], BF16)
            make_identity(nc, identb[:])
            ones = consts.tile([1, 128], F32)
            nc.vector.memset(ones[:], 1.0)

            # bh2 pieces: per-sample-weight biases (fixed matrices / rows)
            Bh2W1 = wts.tile([128, 2, M], F32)  # bh2[:i0] as [D, M]
            nc.sync.dma_start(
                out=Bh2W1[:], in_=bh2[0:I0].rearrange("(dc p m) -> p dc m", p=128, m=M)
            )
            Bh2W2 = wts.tile([128, 2, D], BF16)  # bh2[i1:i2] as [M, D]
            nc.gpsimd.dma_start(
                out=Bh2W2[:], in_=bh2[I1:I2].rearrange("(mc p d) -> p mc d", p=128, d=D)
            )
            Wh2b1 = wts.tile([128, 2, M], F32)  # Wh2[:, i0:i1] (k-major)
            nc.sync.dma_start(
                out=Wh2b1[:], in_=Wh2[:, I0:I1].rearrange("(cc p) m -> p cc m", p=128)
            )
            Wh2b2 = wts.tile([128, 2, D], F32)  # Wh2[:, i2:]
            nc.sync.dma_start(
                out=Wh2b2[:], in_=Wh2[:, I2:NPARAMS].rearrange("(cc p) m -> p cc m", p=128)
            )
            bh2b1 = wts.tile([1, M], F32)
            nc.sync.dma_start(out=bh2b1[:], in_=bh2[None, I0:I1])
            bh2b2 = wts.tile([1, D], F32)
            nc.sync.dma_start(out=bh2b2[:], in_=bh2[None, I2:NPARAMS])

            # ---- transposes of x / context ----
            xT = acts.tile([OBS, BS], F32)
            ctxT = acts.tile([CTX, BS], F32)
            for hb in range(2):
                pt = pmisc.tile([128, 256], F32, tag="pm")
                nc.tensor.transpose(pt[0:OBS, 0:128], xsb[:, hb, :], ident[:])
                nc.vector.tensor_copy(xT[:, hb * 128:(hb + 1) * 128], pt[0:OBS, 0:128])
                pt2 = pmisc.tile([128, 256], F32, tag="pm")
                nc.tensor.transpose(pt2[0:CTX, 0:128], ctxsb[:, hb, :], ident[:])
                nc.vector.tensor_copy(ctxT[:, hb * 128:(hb + 1) * 128], pt2[0:CTX, 0:128])

            # ---- main MLP: hT = relu(W1.T @ relu(W0.T @ xT + b0) + b1) ----
            h1T = acts.tile([128, 2, BS], F32)
            for dc in range(2):
                ph = pmisc.tile([128, 256], F32, tag="pm")
                nc.tensor.matmul(
                    ph[:], W0sb[:, dc * 128:(dc + 1) * 128], xT[:], start=True, stop=True
                )
                nc.scalar.activation(h1T[:, dc, :], ph[:], AF.Relu, bias=b0sb[:, dc:dc + 1])
            hT = acts.tile([128, 2, BS], F32)
            for dc2 in range(2):
                ph = pmisc.tile([128, 256], F32, tag="pm")
                nc.tensor.matmul(
                    ph[:], W1sb[:, 0, dc2 * 128:(dc2 + 1) * 128], h1T[:, 0, :],
                    start=True, stop=False,
                )
                nc.tensor.matmul(
                    ph[:], W1sb[:, 1, dc2 * 128:(dc2 + 1) * 128], h1T[:, 1, :],
                    start=False, stop=True,
                )
                nc.scalar.activation(hT[:, dc2, :], ph[:], AF.Relu, bias=b1sb[:, dc2:dc2 + 1])
            hTb = acts.tile([128, 2, BS], BF16)  # bf16 lhsT for the stream matmuls
            nc.vector.tensor_copy(hTb[:], hT[:])

            # ---- hypernet first layer: cT = relu(Wh1.T @ ctxT + bh1) ----
            cT = acts.tile([128, 2, BS], F32)
            for cc in range(2):
                ph = pmisc.tile([128, 256], F32, tag="pm")
                nc.tensor.matmul(
                    ph[:], Wh1sb[:, cc * 128:(cc + 1) * 128], ctxT[:], start=True, stop=True
                )
                nc.scalar.activation(cT[:, cc, :], ph[:], AF.Relu, bias=bh1sb[:, cc:cc + 1])

            # ---- c (b-major, per-partition scale operands) ----
            csb = acts.tile([128, 2, H], F32)
            for cc in range(2):
                for hb in range(2):
                    pt = pmisc.tile([128, 256], F32, tag="pm")
                    nc.tensor.transpose(
                        pt[:, 0:128], cT[:, cc, hb * 128:(hb + 1) * 128], ident[:]
                    )
                    nc.vector.tensor_copy(csb[:, hb, cc * 128:(cc + 1) * 128], pt[:, 0:128])
            # ---- h (b-major, for the skip connection) ----
            hsb = acts.tile([128, 2, D], F32)
            for dc in range(2):
                for hb in range(2):
                    pt = pmisc.tile([128, 256], F32, tag="pm")
                    nc.tensor.transpose(
                        pt[:, 0:128], hT[:, dc, hb * 128:(hb + 1) * 128], ident[:]
                    )
                    nc.vector.tensor_copy(hsb[:, hb, dc * 128:(dc + 1) * 128], pt[:, 0:128])

            zTb = None

            def init_mms(dst, hb, layer, start_first):
                """Accumulate the bias (+ skip) terms for one batch-half into dst."""
                sl = slice(hb * 128, (hb + 1) * 128)
                s = start_first
                if layer == 0:
                    nc.tensor.matmul(dst, hT[:, 0, sl], Bh2W1[:, 0, :], start=s, stop=False)
                    nc.tensor.matmul(dst, hT[:, 1, sl], Bh2W1[:, 1, :], start=False, stop=False)
                    nc.tensor.matmul(dst, cT[:, 0, sl], Wh2b1[:, 0, :], start=False, stop=False)
                    nc.tensor.matmul(dst, cT[:, 1, sl], Wh2b1[:, 1, :], start=False, stop=False)
                    nc.tensor.matmul(dst, ones[:], bh2b1[:], start=False, stop=(hb == 0))
                else:
                    nc.tensor.matmul(dst, zTb[:, 0, sl], Bh2W2[:, 0, :], start=s, stop=False)
                    nc.tensor.matmul(dst, zTb[:, 1, sl], Bh2W2[:, 1, :], start=False, stop=False)
                    nc.tensor.matmul(dst, cT[:, 0, sl], Wh2b2[:, 0, :], start=False, stop=False)
                    nc.tensor.matmul(dst, cT[:, 1, sl], Wh2b2[:, 1, :], start=False, stop=False)
                    nc.tensor.matmul(dst, ones[:], bh2b2[:], start=False, stop=False)
                    nc.tensor.matmul(dst, ident[:], hsb[:, hb, :], start=False, stop=(hb == 0))

            def stream_layer(layer):
                """One per-sample layer; returns (sbuf acc half0, psum acc half1)."""
                lhsTb = hTb if layer == 0 else zTb
                col0 = 0 if layer == 0 else I1
                # half 0: fused DVE scalar_tensor_tensor into SBUF, seeded from
                # a standalone PSUM init group copied over by ACT.
                pi = pmisc.tile([128, 256], F32, tag="pm")
                init_mms(pi[:], 0, layer, start_first=True)
                acc_sb = accs.tile([128, 256], F32, tag=f"zsb{layer}")
                nc.scalar.activation(acc_sb[:], pi[:], AF.Copy)
                # half 1: single contiguous PSUM accumulation group
                # (init matmuls + 256 identity-matmul adds of bf16 tmps).
                acc_ps = paccp.tile([128, 256], F32, tag="zps")
                init_mms(acc_ps[:], 1, layer, start_first=True)

                for g in range(NGROUPS):
                    wt = wh2s.tile([128, G, 2, 256], BF16, tag="wt")
                    for ch in range(2):
                        lo = col0 + ch * 128 * 256
                        nc.gpsimd.dma_start(
                            out=wt[:, :, ch, :],
                            in_=Wh2[g * G:(g + 1) * G, lo:lo + 128 * 256]
                            .rearrange("kk (p m) -> p kk m", p=128, m=256),
                        )
                    for kk in range(G):
                        k = g * G + kk
                        pk = pkp.tile([128, 2, 256], F32, tag="pk")
                        for hb in range(2):
                            sl = slice(hb * 128, (hb + 1) * 128)
                            nc.tensor.matmul(pk[:, hb, :], lhsTb[:, 0, sl],
                                             wt[:, kk, 0, :], start=True, stop=False)
                            nc.tensor.matmul(pk[:, hb, :], lhsTb[:, 1, sl],
                                             wt[:, kk, 1, :], start=False, stop=True)
                        nc.vector.scalar_tensor_tensor(
                            acc_sb[:], pk[:, 0, :], csb[:, 0, k:k + 1],
                            acc_sb[:], op0=ALU.mult, op1=ALU.add,
                        )
                        tmp = tmps.tile([128, 256], BF16, tag="tmp")
                        nc.scalar.activation(
                            tmp[:], pk[:, 1, :], AF.Copy, scale=csb[:, 1, k:k + 1]
                        )
                        nc.tensor.matmul(acc_ps[:], identb[:], tmp[:],
                                         start=False, stop=(k == H - 1))
                return acc_sb, acc_ps

            # ---- layer 1 ----
            z_sb, z_ps = stream_layer(0)
            zrel = acts.tile([128, 2, M], F32)
            nc.scalar.activation(zrel[:, 0, :], z_sb[:], AF.Relu)
            nc.scalar.activation(zrel[:, 1, :], z_ps[:], AF.Relu)
            zTb = acts.tile([128, 2, BS], BF16)
            for mc in range(2):
                for hb in range(2):
                    pt = pmisc.tile([128, 256], F32, tag="pm")
                    nc.tensor.transpose(
                        pt[:, 0:128], zrel[:, hb, mc * 128:(mc + 1) * 128], ident[:]
                    )
                    nc.vector.tensor_copy(zTb[:, mc, hb * 128:(hb + 1) * 128], pt[:, 0:128])

            # ---- layer 2 ----
            q_sb, q_ps = stream_layer(1)
            orel = acts.tile([128, 2, D], F32)
            nc.scalar.activation(orel[:, 0, :], q_sb[:], AF.Relu)
            nc.scalar.activation(orel[:, 1, :], q_ps[:], AF.Relu)
            nc.sync.dma_start(
                out=out[:, :].rearrange("(hb p) d -> p hb d", p=128), in_=orel[:]
            )

    nc.compile()
    return nc


def _in_maps(inputs):
    full = {k: np.ascontiguousarray(np.asarray(v, dtype=np.float32)) for k, v in inputs.items()}
    maps = []
    for i in range(NCORES):
        m = dict(full)
        m["x"] = full["x"][i * BS:(i + 1) * BS]
        m["context"] = full["context"][i * BS:(i + 1) * BS]
        maps.append(m)
    return maps


def _get_nc():
    global _CACHED_NC
    if _CACHED_NC is None:
        _CACHED_NC = build_nc()
    return _CACHED_NC


def run_spmd(inputs, trace=False):
    from concourse.bass_utils import run_bass_kernel_spmd

    nc = _get_nc()
    res = run_bass_kernel_spmd(nc, _in_maps(inputs), list(range(NCORES)), trace=trace)
    out = np.concatenate([res.results[i]["out"] for i in range(NCORES)], axis=0)
    return out, res


def kernel(**inputs) -> np.ndarray:
    out, _ = run_spmd(inputs, trace=False)
    return out


# revision 6
# speedup vs baseline: 1.2706x; 1.2706x over previous
"""Trainium2 Bass kernel for the hypernet-MoE model (nn_BaseModel_53455162966557).

Math (per sample b):
    h  = relu(relu(x @ W0 + b0) @ W1 + b1)                    [B, D]
    c  = relu(context @ Wh1 + bh1)                            [B, H]
    flat = c @ Wh2 + bh2                                      [B, NPARAMS]
    z  = relu(einsum(h, flat[:, :i0] as [D, M]) + flat[:, i0:i1])
    z2 = einsum(z, flat[:, i1:i2] as [M, D]) + flat[:, i2:]
    out = relu(h + z2)

Key restructuring: flat is never materialized.  For each hypernet unit k,
    A1_k = h @ Wh2[k, :i0].reshape(D, M)        (tensor engine, bf16)
    z    = relu(sum_k c[:, k] * A1_k + h @ Bh2W1 + b1m)
and symmetrically for the second per-sample layer with zT.  Wh2 (135 MB)
streams through SBUF exactly once per core (cast to bf16 in the DMA).

The per-k scale+accumulate is split across three engines so no single one
binds: for each k, one batch-half uses a fused DVE scalar_tensor_tensor
(fp32), the other uses an ACT per-partition-scale copy to a bf16 tmp
followed by a PE identity-matmul accumulate into a PSUM accumulator.
The halves rotate with k parity.

Sharding: pure data parallel, batch 2048 -> 8 cores x 256.
"""

import ml_dtypes
import numpy as np

import concourse.bass as bass
import concourse.tile as tile
from concourse import bacc, mybir
from concourse.masks import make_identity

F32 = mybir.dt.float32
BF16 = mybir.dt.bfloat16
AF = mybir.ActivationFunctionType
ALU = mybir.AluOpType

B, OBS, CTX, D, M, H = 2048, 64, 16, 256, 256, 256
NCORES = 8
BS = B // NCORES  # 256 rows per core
I0 = D * M  # 65536
I1 = I0 + M  # 65792
I2 = I1 + M * D  # 131328
NPARAMS = I2 + D  # 131584
G = 16  # Wh2 rows (k values) per DMA group (bf16 staged, contiguous)
NGROUPS = H // G

_CACHED_NC = None


def build_nc():
    nc = bacc.Bacc("TRN2", target_bir_lowering=False, debug=False)

    x = nc.dram_tensor("x", [BS, OBS], F32, kind="ExternalInput")
    ctx_in = nc.dram_tensor("context", [BS, CTX], F32, kind="ExternalInput")
    W0 = nc.dram_tensor("W0", [OBS, D], F32, kind="ExternalInput")
    b0 = nc.dram_tensor("b0", [D], F32, kind="ExternalInput")
    W1 = nc.dram_tensor("W1", [D, D], F32, kind="ExternalInput")
    b1 = nc.dram_tensor("b1", [D], F32, kind="ExternalInput")
    Wh1 = nc.dram_tensor("Wh1", [CTX, H], F32, kind="ExternalInput")
    bh1 = nc.dram_tensor("bh1", [H], F32, kind="ExternalInput")
    Wh2s = nc.dram_tensor("Wh2s", [2, NGROUPS, 2, G * 128 * 256], BF16,
                          kind="ExternalInput")
    Wh2e = nc.dram_tensor("Wh2e", [H, 512], F32, kind="ExternalInput")
    bh2 = nc.dram_tensor("bh2", [NPARAMS], F32, kind="ExternalInput")
    out = nc.dram_tensor("out", [BS, D], F32, kind="ExternalOutput")

    with tile.TileContext(nc) as tc:
        with (
            tc.tile_pool(name="consts", bufs=1) as consts,
            tc.tile_pool(name="wts", bufs=1) as wts,
            tc.tile_pool(name="acts", bufs=1) as acts,
            tc.tile_pool(name="accs", bufs=1) as accs,
            tc.tile_pool(name="tmps", bufs=4) as tmps,
            tc.tile_pool(name="wh2s", bufs=4) as wh2s,
            tc.tile_pool(name="pk", bufs=5, space="PSUM") as pkp,
            tc.tile_pool(name="pacc", bufs=1, space="PSUM") as paccp,
            tc.tile_pool(name="pmisc", bufs=2, space="PSUM") as pmisc,
        ):
            # ---- input activations first: they head the critical path ----
            xsb = acts.tile([128, 2, OBS], F32)
            nc.sync.dma_start(out=xsb[:], in_=x[:, :].rearrange("(hb p) o -> p hb o", p=128))
            ctxsb = acts.tile([128, 2, CTX], F32)
            nc.sync.dma_start(out=ctxsb[:], in_=ctx_in[:, :].rearrange("(hb p) o -> p hb o", p=128))
            W0sb = wts.tile([OBS, D], F32)
            nc.sync.dma_start(out=W0sb[:], in_=W0[:])
            W1sb = wts.tile([128, 2, D], F32)
            nc.sync.dma_start(out=W1sb[:], in_=W1[:, :].rearrange("(cc p) j -> p cc j", p=128))
            Wh1sb = wts.tile([CTX, H], F32)
            nc.sync.dma_start(out=Wh1sb[:], in_=Wh1[:])
            b0sb = wts.tile([128, 2], F32)
            nc.sync.dma_start(out=b0sb[:], in_=b0[:].rearrange("(cc p) -> p cc", p=128))
            b1sb = wts.tile([128, 2], F32)
            nc.sync.dma_start(out=b1sb[:], in_=b1[:].rearrange("(cc p) -> p cc", p=128))
            bh1sb = wts.tile([128, 2], F32)
            nc.sync.dma_start(out=bh1sb[:], in_=bh1[:].rearrange("(cc p) -> p cc", p=128))

            ident = consts.tile([128, 128], F32)
            make_identity(nc, ident[:])
            identb = consts.tile([128, 128], BF16)
            make_identity(nc, identb[:])
            ones = consts.tile([1, 128], F32)
            nc.vector.memset(ones[:], 1.0)

            # bh2 pieces: per-sample-weight biases (fixed matrices / rows)
            Bh2W1 = wts.tile([128, 2, M], F32)  # bh2[:i0] as [D, M]
            nc.sync.dma_start(
                out=Bh2W1[:], in_=bh2[0:I0].rearrange("(dc p m) -> p dc m", p=128, m=M)
            )
            Bh2W2 = wts.tile([128, 2, D], BF16)  # bh2[i1:i2] as [M, D]
            nc.gpsimd.dma_start(
                out=Bh2W2[:], in_=bh2[I1:I2].rearrange("(mc p d) -> p mc d", p=128, d=D)
            )  # dtype cast in the (SWDGE) DMA
            Wh2b1 = wts.tile([128, 2, M], F32)  # Wh2[:, i0:i1] (k-major)
            nc.sync.dma_start(
                out=Wh2b1[:], in_=Wh2e[:, 0:256].rearrange("(cc p) m -> p cc m", p=128)
            )
            Wh2b2 = wts.tile([128, 2, D], F32)  # Wh2[:, i2:]
            nc.sync.dma_start(
                out=Wh2b2[:], in_=Wh2e[:, 256:512].rearrange("(cc p) m -> p cc m", p=128)
            )
            bh2b1 = wts.tile([1, M], F32)
            nc.sync.dma_start(out=bh2b1[:], in_=bh2[None, I0:I1])
            bh2b2 = wts.tile([1, D], F32)
            nc.sync.dma_start(out=bh2b2[:], in_=bh2[None, I2:NPARAMS])

            # ---- transposes of x / context ----
            xT = acts.tile([OBS, BS], F32)
            ctxT = acts.tile([CTX, BS], F32)
            for hb in range(2):
                pt = pmisc.tile([128, 256], F32, tag="pm")
                nc.tensor.transpose(pt[0:OBS, 0:128], xsb[:, hb, :], ident[:])
                nc.vector.tensor_copy(xT[:, hb * 128:(hb + 1) * 128], pt[0:OBS, 0:128])
                pt2 = pmisc.tile([128, 256], F32, tag="pm")
                nc.tensor.transpose(pt2[0:CTX, 0:128], ctxsb[:, hb, :], ident[:])
                nc.vector.tensor_copy(ctxT[:, hb * 128:(hb + 1) * 128], pt2[0:CTX, 0:128])

            # ---- main MLP: hT = relu(W1.T @ relu(W0.T @ xT + b0) + b1) ----
            h1T = acts.tile([128, 2, BS], F32)
            for dc in range(2):
                ph = pmisc.tile([128, 256], F32, tag="pm")
                nc.tensor.matmul(
                    ph[:], W0sb[:, dc * 128:(dc + 1) * 128], xT[:], start=True, stop=True
                )
                nc.scalar.activation(h1T[:, dc, :], ph[:], AF.Relu, bias=b0sb[:, dc:dc + 1])
            hT = acts.tile([128, 2, BS], F32)
            for dc2 in range(2):
                ph = pmisc.tile([128, 256], F32, tag="pm")
                nc.tensor.matmul(
                    ph[:], W1sb[:, 0, dc2 * 128:(dc2 + 1) * 128], h1T[:, 0, :],
                    start=True, stop=False,
                )
                nc.tensor.matmul(
                    ph[:], W1sb[:, 1, dc2 * 128:(dc2 + 1) * 128], h1T[:, 1, :],
                    start=False, stop=True,
                )
                nc.scalar.activation(hT[:, dc2, :], ph[:], AF.Relu, bias=b1sb[:, dc2:dc2 + 1])
            hTb = acts.tile([128, 2, BS], BF16)  # bf16 lhsT for the stream matmuls
            nc.vector.tensor_copy(hTb[:], hT[:])

            # ---- hypernet first layer: cT = relu(Wh1.T @ ctxT + bh1) ----
            cT = acts.tile([128, 2, BS], F32)
            for cc in range(2):
                ph = pmisc.tile([128, 256], F32, tag="pm")
                nc.tensor.matmul(
                    ph[:], Wh1sb[:, cc * 128:(cc + 1) * 128], ctxT[:], start=True, stop=True
                )
                nc.scalar.activation(cT[:, cc, :], ph[:], AF.Relu, bias=bh1sb[:, cc:cc + 1])

            # ---- c (b-major, per-partition scale operands) ----
            csb = acts.tile([128, 2, H], F32)
            for cc in range(2):
                for hb in range(2):
                    pt = pmisc.tile([128, 256], F32, tag="pm")
                    nc.tensor.transpose(
                        pt[:, 0:128], cT[:, cc, hb * 128:(hb + 1) * 128], ident[:]
                    )
                    nc.vector.tensor_copy(csb[:, hb, cc * 128:(cc + 1) * 128], pt[:, 0:128])
            # ---- h (b-major, for the skip connection) ----
            hsb = acts.tile([128, 2, D], F32)
            for dc in range(2):
                for hb in range(2):
                    pt = pmisc.tile([128, 256], F32, tag="pm")
                    nc.tensor.transpose(
                        pt[:, 0:128], hT[:, dc, hb * 128:(hb + 1) * 128], ident[:]
                    )
                    nc.vector.tensor_copy(hsb[:, hb, dc * 128:(dc + 1) * 128], pt[:, 0:128])

            zTb = None

            def init_mms(dst, hb, layer, start_first):
                """Accumulate the bias (+ skip) terms for one batch-half into dst."""
                sl = slice(hb * 128, (hb + 1) * 128)
                s = start_first
                if layer == 0:
                    nc.tensor.matmul(dst, hT[:, 0, sl], Bh2W1[:, 0, :], start=s, stop=False)
                    nc.tensor.matmul(dst, hT[:, 1, sl], Bh2W1[:, 1, :], start=False, stop=False)
                    nc.tensor.matmul(dst, cT[:, 0, sl], Wh2b1[:, 0, :], start=False, stop=False)
                    nc.tensor.matmul(dst, cT[:, 1, sl], Wh2b1[:, 1, :], start=False, stop=False)
                    nc.tensor.matmul(dst, ones[:], bh2b1[:], start=False, stop=(hb == 0))
                else:
                    nc.tensor.matmul(dst, zTb[:, 0, sl], Bh2W2[:, 0, :], start=s, stop=False)
                    nc.tensor.matmul(dst, zTb[:, 1, sl], Bh2W2[:, 1, :], start=False, stop=False)
                    nc.tensor.matmul(dst, cT[:, 0, sl], Wh2b2[:, 0, :], start=False, stop=False)
                    nc.tensor.matmul(dst, cT[:, 1, sl], Wh2b2[:, 1, :], start=False, stop=False)
                    nc.tensor.matmul(dst, ones[:], bh2b2[:], start=False, stop=False)
                    nc.tensor.matmul(dst, ident[:], hsb[:, hb, :], start=False, stop=(hb == 0))

            def stream_layer(layer):
                """One per-sample layer; returns (sbuf acc half0, psum acc half1)."""
                lhsTb = hTb if layer == 0 else zTb
                col0 = 0 if layer == 0 else I1
                # half 0: fused DVE scalar_tensor_tensor into SBUF, seeded from
                # a standalone PSUM init group copied over by ACT.
                pi = pmisc.tile([128, 256], F32, tag="pm")
                init_mms(pi[:], 0, layer, start_first=True)
                acc_sb = accs.tile([128, 256], F32, tag=f"zsb{layer}")
                nc.scalar.activation(acc_sb[:], pi[:], AF.Copy)
                # half 1: single contiguous PSUM accumulation group
                # (init matmuls + 256 identity-matmul adds of bf16 tmps).
                acc_ps = paccp.tile([128, 256], F32, tag="zps")
                init_mms(acc_ps[:], 1, layer, start_first=True)

                for g in range(NGROUPS):
                    wt = wh2s.tile([128, 2, G, 256], BF16, tag="wt")
                    for ch in range(2):
                        nc.sync.dma_start(
                            out=wt[:, ch, :, :],
                            in_=Wh2s[layer, g, ch, :]
                            .rearrange("(p f) -> p f", p=128),
                        )
                    for pp in range(G // 2):
                        k0 = g * G + 2 * pp
                        pkA = pkp.tile([128, 2, 256], F32, tag="pk")
                        pkB = pkp.tile([128, 2, 256], F32, tag="pk")
                        for hb, pkt in ((0, pkA), (1, pkB)):
                            sl = slice(hb * 128, (hb + 1) * 128)
                            nc.tensor.matmul(pkt[:, :, :], lhsTb[:, 0, sl],
                                             wt[:, 0, 2 * pp:2 * pp + 2, :],
                                             start=True, stop=False)
                            nc.tensor.matmul(pkt[:, :, :], lhsTb[:, 1, sl],
                                             wt[:, 1, 2 * pp:2 * pp + 2, :],
                                             start=False, stop=True)
                        tmp = tmps.tile([128, 2, 256], BF16, tag="tmp")
                        for kk in range(2):
                            k = k0 + kk
                            nc.vector.scalar_tensor_tensor(
                                acc_sb[:], pkA[:, kk, :], csb[:, 0, k:k + 1],
                                acc_sb[:], op0=ALU.mult, op1=ALU.add,
                            )
                            nc.scalar.activation(
                                tmp[:, kk, :], pkB[:, kk, :], AF.Copy,
                                scale=csb[:, 1, k:k + 1],
                            )
                            nc.tensor.matmul(acc_ps[:], identb[:], tmp[:, kk, :],
                                             start=False, stop=(k == H - 1))
                return acc_sb, acc_ps

            # ---- layer 1 ----
            z_sb, z_ps = stream_layer(0)
            zrel = acts.tile([128, 2, M], F32)
            nc.scalar.activation(zrel[:, 0, :], z_sb[:], AF.Relu)
            nc.scalar.activation(zrel[:, 1, :], z_ps[:], AF.Relu)
            zTb = acts.tile([128, 2, BS], BF16)
            for mc in range(2):
                for hb in range(2):
                    pt = pmisc.tile([128, 256], F32, tag="pm")
                    nc.tensor.transpose(
                        pt[:, 0:128], zrel[:, hb, mc * 128:(mc + 1) * 128], ident[:]
                    )
                    nc.vector.tensor_copy(zTb[:, mc, hb * 128:(hb + 1) * 128], pt[:, 0:128])

            # ---- layer 2 ----
            q_sb, q_ps = stream_layer(1)
            orel = acts.tile([128, 2, D], F32)
            nc.scalar.activation(orel[:, 0, :], q_sb[:], AF.Relu)
            nc.scalar.activation(orel[:, 1, :], q_ps[:], AF.Relu)
            nc.sync.dma_start(
                out=out[:, :].rearrange("(hb p) d -> p hb d", p=128), in_=orel[:]
            )

    nc.compile()
    return nc


def _stage_wh2(Wh2):
    """Host-side staging: bf16 cast + the exact tile layout the stream DMAs
    read, so every group transfer is one fully contiguous block."""
    W1p = Wh2[:, :I0].reshape(H, 2, 128, 256)       # k, ch, p, m
    W2p = Wh2[:, I1:I2].reshape(H, 2, 128, 256)
    s = np.stack([W1p, W2p])                        # l, k, ch, p, m
    s = s.reshape(2, NGROUPS, G, 2, 128, 256).transpose(0, 1, 3, 4, 2, 5)
    Wh2s = np.ascontiguousarray(s).astype(ml_dtypes.bfloat16)
    Wh2s = Wh2s.reshape(2, NGROUPS, 2, G * 128 * 256)
    Wh2e = np.ascontiguousarray(
        np.concatenate([Wh2[:, I0:I1], Wh2[:, I2:]], axis=1)
    )
    return Wh2s, Wh2e


def _in_maps(inputs):
    full = {k: np.ascontiguousarray(np.asarray(v, dtype=np.float32)) for k, v in inputs.items()}
    Wh2s, Wh2e = _stage_wh2(full.pop("Wh2"))
    full["Wh2s"] = Wh2s
    full["Wh2e"] = Wh2e
    maps = []
    for i in range(NCORES):
        m = dict(full)
        m["x"] = full["x"][i * BS:(i + 1) * BS]
        m["context"] = full["context"][i * BS:(i + 1) * BS]
        maps.append(m)
    return maps


def _get_nc():
    global _CACHED_NC
    if _CACHED_NC is None:
        _CACHED_NC = build_nc()
    return _CACHED_NC


def run_spmd(inputs, trace=False):
    from concourse.bass_utils import run_bass_kernel_spmd

    nc = _get_nc()
    res = run_bass_kernel_spmd(nc, _in_maps(inputs), list(range(NCORES)), trace=trace)
    out = np.concatenate([res.results[i]["out"] for i in range(NCORES)], axis=0)
    return out, res


def kernel(**inputs) -> np.ndarray:
    out, _ = run_spmd(inputs, trace=False)
    return out
